# revision 3
# baseline (speedup 1.0000x reference)
"""Trainium2 Bass kernel for nn_CSFlow (RAFT correlation pyramid lookup), v2.

Changes vs v1 baseline:
  - Sparse slab matmul: queries are grouped into 32x4 pixel blocks; each
    block's 10x10 lookup windows (all levels) fall inside a narrow x-range
    of the (pooled) fmap2 grid, so the per-tile matmul only covers a static
    x-slab of each pyramid level (~2.4x fewer columns than the full volume).
    Block-to-tile mapping keeps the w-block a pure function of the tile
    index so the SPMD program (shared by all 8 cores) can hardcode slab
    offsets; slab bounds are computed from the actual coords at build time
    and the build cache is keyed on them.
  - bf16 matmul inputs (fp32 accumulate in PSUM): full 1 col/cycle PE rate
    and half the prologue HBM traffic.
  - No on-device output transpose: blend results go out as [tile, query,
    chan] fp16 and the host reorders/casts. Kills the PE transpose tail.
  - Prologue loads are chunked so early matmuls overlap the tail of the
    input DMA.

Everything else (HBM scratch + banded indirect gather + separable blends
with masks folded into stage-1 weights) follows v1.
"""

import numpy as np

import concourse.bass as bass
import concourse.mybir as mybir
import concourse.tile as tile
from concourse import bacc
from concourse.bass_utils import run_bass_kernel_spmd

# problem shape (hardcoded per harness contract)
B, D, H, W = 2, 256, 48, 160
NCORES = 8
P = 128
BW, BH = 32, 4                    # pixel block per tile (queries = 32*4 = 128)
NWB, NHB = W // BW, H // BH       # 5 x 12 blocks per batch
NT = (B * H * W) // (NCORES * P)  # 15 tiles per core
QPC = NT * P
NLVL = 4
LH = [48, 24, 12, 6]
LW = [160, 80, 40, 20]
LHW = [LH[i] * LW[i] for i in range(NLVL)]
LOFF = [0, 7680, 9600, 10080]
NPOS = 10200
XMAJ = [True, True, True, False]
ST = [48, 24, 12, 20]             # band inner-axis size
BAND = [9 * s + 10 for s in ST]   # 442 226 118 190
# section stride must fit the 10*s blend view (only 9s+10 elems are read)
BOFF = [0, 480, 720, 840]
BTOT = 1040
HEAD = 512

F16 = mybir.dt.float16
BF16 = mybir.dt.bfloat16
F32 = mybir.dt.float32
I32 = mybir.dt.int32

PSUM_CHUNK = 1024
MM_CHUNK = 512
NCH = NLVL * 81                   # 324 output channels


def _chunks(total, step):
    return [(o, min(step, total - o)) for o in range(0, total, step)]


def block_of(c, t):
    """core, tile -> (batch, h0, w0) of its 32x4 pixel block."""
    b = c // (NCORES // B)
    cl = c % (NCORES // B)
    w0 = (t % NWB) * BW
    h0 = (cl * (NHB // (NCORES // B)) + t // NWB) * BH
    return b, h0, w0


def build_nc(slabs, repeat=1, repeat_all=False, do_write=True, do_gather=True,
             do_blend=True, do_mm=True, do_copy=True, copy_mod=0, ps_bufs=3,
             gfuse=False, ck_bufs=12, band_bufs=6, ot_bufs=4, pool_blend=False):
    """slabs: tuple of NT tuples of NLVL (ox, sx) pairs, in level-l x units.
    Level 3 must be (0, LW[3]) (full, y-major)."""
    nc = bacc.Bacc("TRN2", target_bir_lowering=False, debug=False)

    # per-tile scratch geometry (level 3 stores the full map, y-major)
    scols = [[slabs[t][l][1] * LH[l] for l in range(3)] + [LHW[3]]
             for t in range(NT)]
    soff = [[HEAD + P * sum(scols[t][:l]) for l in range(NLVL)]
            for t in range(NT)]
    TAIL = 1024
    scrn = [HEAD + P * sum(scols[t]) + TAIL for t in range(NT)]
    boff = [0, 480, 960, 1440] if gfuse else BOFF
    btot = 1920 if gfuse else BTOT

    f1t = nc.dram_tensor("f1t", [2, P, QPC], BF16, kind="ExternalInput")
    f2t = nc.dram_tensor("f2t", [2, P, NPOS], BF16, kind="ExternalInput")
    idxt = nc.dram_tensor("idxt", [P, NLVL * NT], I32, kind="ExternalInput")
    wgtt = nc.dram_tensor("wgtt", [P, NLVL * NT * 2], F32, kind="ExternalInput")
    my0t = nc.dram_tensor("my0t", [P, NLVL * NT * 90], F16, kind="ExternalInput")
    my1t = nc.dram_tensor("my1t", [P, NLVL * NT * 90], F16, kind="ExternalInput")
    outp = nc.dram_tensor("outp", [NT, P, NCH], F16, kind="ExternalOutput")

    with tile.TileContext(nc) as tc:
        with (
            tc.tile_pool(name="dram", bufs=1, space="DRAM") as dpool,
            tc.tile_pool(name="const", bufs=1) as cpool,
            tc.tile_pool(name="corrchunk", bufs=ck_bufs) as ckpool,
            tc.tile_pool(name="bands", bufs=band_bufs) as bpool,
            tc.tile_pool(name="blend1", bufs=8) as t1pool,
            tc.tile_pool(name="blend2", bufs=4) as t2pool,
            tc.tile_pool(name="otile", bufs=ot_bufs) as opool,
            tc.tile_pool(name="psum", bufs=ps_bufs, space="PSUM") as pspool,
        ):
            import contextlib

            octx = (tc.For_i(0, repeat, 1) if repeat > 1 and repeat_all
                    else contextlib.nullcontext())
            octx.__enter__()

            scrt = [dpool.tile([scrn[t]], F16, name=f"scrt{t}") for t in range(NT)]

            # ---- inputs: chunked, need-ordered loads split across both
            # HWDGE queues; scratch guards go on the Pool queue so they
            # never delay the matmul-feeding loads. First MM needs only
            # f1[k0] + f2[k0] chunk 0 (~1 MB across 2 queues). ----
            zguard = cpool.tile([1, TAIL], F16)
            nc.vector.memset(zguard[:], 0.0)
            for t in range(NT):
                nc.gpsimd.dma_start(scrt[t][0:HEAD].unsqueeze(0),
                                    zguard[0:1, 0:HEAD])
                nc.gpsimd.dma_start(
                    scrt[t][scrn[t] - TAIL : scrn[t]].unsqueeze(0), zguard[0:1, :]
                )

            f1sb = cpool.tile([P, 2 * QPC], BF16)
            f2sb0 = cpool.tile([P, NPOS], BF16)
            f2sb1 = cpool.tile([P, NPOS], BF16)
            idx_sb = cpool.tile([P, NLVL * NT], I32)
            wgt_sb = cpool.tile([P, NLVL * NT * 2], F32)
            my0_sb = cpool.tile([P, NLVL * NT * 90], F16)
            my1_sb = cpool.tile([P, NLVL * NT * 90], F16)

            l0c = _chunks(LHW[0], LHW[0] // 4)
            nc.scalar.dma_start(f1sb[:, 0:QPC], f1t[0])
            for i, (coff, csz) in enumerate(l0c):
                nc.sync.dma_start(f2sb0[:, coff : coff + csz],
                                  f2t[0][:, coff : coff + csz])
                eng = nc.scalar if i == 0 else nc.sync
                eng.dma_start(f2sb1[:, coff : coff + csz],
                              f2t[1][:, coff : coff + csz])
                if i == 0:
                    nc.scalar.dma_start(f1sb[:, QPC : 2 * QPC], f1t[1])
            nc.sync.dma_start(f2sb0[:, LOFF[1] : NPOS], f2t[0][:, LOFF[1] : NPOS])
            nc.scalar.dma_start(f2sb1[:, LOFF[1] : NPOS], f2t[1][:, LOFF[1] : NPOS])
            nc.scalar.dma_start(idx_sb[:], idxt[:])
            nc.scalar.dma_start(wgt_sb[:], wgtt[:])
            nc.sync.dma_start(my0_sb[:], my0t[:])
            nc.scalar.dma_start(my1_sb[:], my1t[:])

            state = {"copy_rr": 0}
            bands = {}
            ots = {}

            def produce(t):
                # === slab matmuls -> psum -> sbuf fp16 -> HBM scratch ===
                for l in range(NLVL):
                    hw = scols[t][l]
                    fbase = LOFF[l] + (slabs[t][l][0] * LH[l] if l < 3 else 0)
                    for coff, csz in _chunks(hw, PSUM_CHUNK):
                        ps = pspool.tile([P, PSUM_CHUNK], F32, name="cps")[:, :csz]
                        for k in range(2 if do_mm else 0):
                            f2sb = f2sb0 if k == 0 else f2sb1
                            for so, ss in _chunks(csz, MM_CHUNK):
                                nc.tensor.matmul(
                                    ps[:, so : so + ss],
                                    f1sb[:, k * QPC + t * P : k * QPC + (t + 1) * P],
                                    f2sb[:, fbase + coff + so : fbase + coff + so + ss],
                                    start=(k == 0),
                                    stop=(k == 1),
                                )
                        if not (do_mm and do_copy):
                            continue
                        ck = ckpool.tile([P, PSUM_CHUNK], F16, name="ck")[:, :csz]
                        if copy_mod and state["copy_rr"] % copy_mod == 0:
                            nc.vector.tensor_copy(ck, ps)
                        else:
                            nc.scalar.copy(ck, ps)
                        state["copy_rr"] += 1
                        if not do_write:
                            continue
                        eng = nc.sync if state["copy_rr"] % 2 == 0 else nc.scalar
                        eng.dma_start(
                            scrt[t][soff[t][l] : soff[t][l] + P * hw]
                            .rearrange("(p x) -> p x", x=hw)[:, coff : coff + csz],
                            ck,
                        )

            def gathers(t):
                # === 4 banded gathers (one per level) ===
                if do_gather and do_write:
                    band = bpool.tile([P, btot], F16, name="band")
                    bands[t] = band
                    if gfuse:
                        nc.gpsimd.indirect_dma_start(
                            out=band[:].rearrange("p (l b) -> p l b", b=480),
                            out_offset=None,
                            in_=scrt[t][:].unsqueeze(1),
                            in_offset=bass.IndirectOffsetOnAxis(
                                ap=idx_sb[:, t * NLVL : t * NLVL + NLVL],
                                axis=0,
                            ),
                            element_offset=0,
                        )
                    else:
                        for l in range(NLVL):
                            nc.gpsimd.indirect_dma_start(
                                out=band[:, boff[l] : boff[l] + BAND[l]],
                                out_offset=None,
                                in_=scrt[t][:].unsqueeze(1),
                                in_offset=bass.IndirectOffsetOnAxis(
                                    ap=idx_sb[:, t * NLVL + l : t * NLVL + l + 1],
                                    axis=0,
                                ),
                                element_offset=0,
                            )

            def consume(t):
                # === separable blends for tile t (gathered a tile ago).
                # Levels 0-1 run wholly on DVE, levels 2-3 wholly on Pool:
                # one engine per level chain (TT,TT,add,sub,stst), no
                # cross-engine ping-pong, ACT keeps the psum copies. ===
                if not (do_gather and do_write and do_blend):
                    return
                # ship tile t-1's finished output first: its blends completed
                # a whole tile ago, so this DMA issue never blocks.
                if t - 1 in ots:
                    nc.scalar.dma_start(outp[t - 1], ots.pop(t - 1)[:])
                band = bands.pop(t)
                ot = opool.tile([P, NCH], F16, name="ot")
                ots[t] = ot
                for l in range(NLVL):
                    eng = nc.vector if (l < 2 or not pool_blend) else nc.gpsimd
                    s_in = ST[l]
                    bw_ = band[:, boff[l] : boff[l] + 10 * s_in].rearrange(
                        "p (r s) -> p r s", s=s_in
                    )
                    g0 = bw_[:, 0:10, 0:9]
                    g1 = bw_[:, 0:10, 1:10]
                    c90 = (l * NT + t) * 90
                    m0 = my0_sb[:, c90 : c90 + 90].rearrange("p (r j) -> p r j", j=9)
                    m1 = my1_sb[:, c90 : c90 + 90].rearrange("p (r j) -> p r j", j=9)
                    t1 = t1pool.tile([P, 90], F32, name="t1")
                    t1v = t1[:].rearrange("p (r j) -> p r j", j=9)
                    t2 = t2pool.tile([P, 90], F32, name="t2")
                    t2v = t2[:].rearrange("p (r j) -> p r j", j=9)
                    eng.tensor_tensor(out=t1v, in0=g0, in1=m0,
                                      op=mybir.AluOpType.mult)
                    eng.tensor_tensor(out=t2v, in0=g1, in1=m1,
                                      op=mybir.AluOpType.mult)
                    eng.tensor_add(out=t1[:], in0=t1[:], in1=t2[:])
                    t1r = t1[:].rearrange("p (r j) -> p r j", j=9)
                    dd = t2pool.tile([P, 81], F32, name="dd")
                    ddv = dd[:].rearrange("p (a j) -> p a j", j=9)
                    c2 = (l * NT + t) * 2
                    ov = ot[:, l * 81 : (l + 1) * 81].rearrange(
                        "p (a j) -> p a j", j=9
                    )
                    eng.tensor_tensor(out=ddv, in0=t1r[:, 1:10, :],
                                      in1=t1r[:, 0:9, :],
                                      op=mybir.AluOpType.subtract)
                    eng.scalar_tensor_tensor(
                        out=ov,
                        in0=ddv,
                        scalar=wgt_sb[:, c2 + 1 : c2 + 2],
                        in1=t1r[:, 0:9, :],
                        op0=mybir.AluOpType.mult,
                        op1=mybir.AluOpType.add,
                    )


            rep_ctx = (tc.For_i(0, repeat, 1) if repeat > 1 and not repeat_all
                       else contextlib.nullcontext())
            with rep_ctx:
                # 1-tile software pipeline; Pool order is blends(t-1)
                # then gathers(t) so blend ops never queue behind a gather
                # that is still waiting on tile t's scratch writes.
                for t in range(NT):
                    produce(t)
                    if t >= 1:
                        consume(t - 1)
                    gathers(t)
                consume(NT - 1)
                for t in sorted(ots):
                    nc.scalar.dma_start(outp[t], ots.pop(t)[:])

            octx.__exit__(None, None, None)

    nc.compile()
    return nc


# ---------------- host side ----------------

def _pool2(x):
    n, c, h, w = x.shape
    return x.reshape(n, c, h // 2, 2, w // 2, 2).mean(axis=(3, 5))


def _query_coords():
    """Global (b, h, w) per (core, tile, partition)."""
    cc, tt, pp = np.meshgrid(np.arange(NCORES), np.arange(NT), np.arange(P),
                             indexing="ij")
    bb = cc // (NCORES // B)
    cl = cc % (NCORES // B)
    w0 = (tt % NWB) * BW
    h0 = (cl * (NHB // (NCORES // B)) + tt // NWB) * BH
    hh = h0 + pp // BW
    ww = w0 + pp % BW
    return bb, hh, ww


_BB, _HH, _WW = _query_coords()


def compute_slabs(coords):
    """Per (tile, level) slab (ox, sx) covering every core's block at that
    tile, from the actual coords. Level 3 is always the full (y-major) map."""
    coords = np.asarray(coords, np.float32)
    cx = coords[_BB, 0, _HH, _WW]  # [NCORES, NT, P]
    slabs = []
    for t in range(NT):
        row = []
        for l in range(3):
            inv = 1.0 / (1 << l)
            x0 = np.floor(cx[:, t] * inv)
            x0 = np.clip(x0, -5, LW[l] + 4)
            lo = max(int(x0.min()) - 4, 0)
            hi = min(int(x0.max()) + 5, LW[l] - 1)
            sx = hi - lo + 1
            row.append((lo, sx))
        row.append((0, LW[3]))
        slabs.append(tuple(row))
    return tuple(slabs)


def _host_prep(fmap1, fmap2, coords, slabs):
    fmap1 = np.asarray(fmap1, np.float32)
    fmap2 = np.asarray(fmap2, np.float32)
    coords = np.asarray(coords, np.float32)
    scale = np.float32(1.0 / np.sqrt(D))
    bf16 = mybir.dt.np(BF16)

    # pooled fmap2 levels, flattened in storage orientation, scaled
    levels = []
    cur = fmap2 * scale
    for l in range(NLVL):
        if XMAJ[l]:
            levels.append(
                np.ascontiguousarray(cur.transpose(0, 1, 3, 2)).reshape(B, D, LHW[l])
            )
        else:
            levels.append(cur.reshape(B, D, LHW[l]))
        if l < NLVL - 1:
            cur = _pool2(cur)
    f2cat = np.concatenate(levels, axis=2).astype(bf16)  # [B, D, NPOS]

    # per-tile scratch geometry (must match build_nc)
    scols = [[slabs[t][l][1] * LH[l] for l in range(3)] + [LHW[3]]
             for t in range(NT)]
    soff = [[HEAD + P * sum(scols[t][:l]) for l in range(NLVL)]
            for t in range(NT)]

    cx = coords[_BB, 0, _HH, _WW]  # [NCORES, NT, P]
    cy = coords[_BB, 1, _HH, _WW]

    idx_all = np.zeros((NCORES, NT, NLVL, P), np.int32)
    wgt_all = np.zeros((NCORES, NT, NLVL, P, 2), np.float32)
    my0_all = np.zeros((NCORES, NT, NLVL, P, 10, 9), np.float16)
    my1_all = np.zeros((NCORES, NT, NLVL, P, 10, 9), np.float16)
    rr = np.arange(10)
    pq = np.arange(P)
    for l in range(NLVL):
        inv = np.float32(1.0 / (1 << l))
        x = cx * inv
        y = cy * inv
        x0 = np.floor(x)
        y0 = np.floor(y)
        wx = (x - x0).astype(np.float32)
        wy = (y - y0).astype(np.float32)
        x0c = np.clip(x0, -5, LW[l] + 4).astype(np.int64)
        y0c = np.clip(y0, -5, LH[l] + 4).astype(np.int64)
        vx = ((x0[..., None] + rr - 4) >= 0) & ((x0[..., None] + rr - 4) <= LW[l] - 1)
        vy = ((y0[..., None] + rr - 4) >= 0) & ((y0[..., None] + rr - 4) <= LH[l] - 1)
        if XMAJ[l]:
            for t in range(NT):
                ox = slabs[t][l][0]
                idx_all[:, t, l] = (
                    soff[t][l] + pq[None, :] * scols[t][l]
                    + (x0c[:, t] - 4 - ox) * LH[l] + (y0c[:, t] - 4)
                ).astype(np.int32)
            wgt_all[:, :, l, :, 0] = 1.0 - wx
            wgt_all[:, :, l, :, 1] = wx
            m0 = vx[..., None] & vy[..., None, 0:9]
            m1 = vx[..., None] & vy[..., None, 1:10]
            my0_all[:, :, l] = m0 * (1.0 - wy)[..., None, None]
            my1_all[:, :, l] = m1 * wy[..., None, None]
        else:
            for t in range(NT):
                idx_all[:, t, l] = (
                    soff[t][l] + pq[None, :] * scols[t][l]
                    + (y0c[:, t] - 4) * LW[l] + (x0c[:, t] - 4)
                ).astype(np.int32)
            wgt_all[:, :, l, :, 0] = 1.0 - wy
            wgt_all[:, :, l, :, 1] = wy
            m0 = vy[..., None] & vx[..., None, 0:9]
            m1 = vy[..., None] & vx[..., None, 1:10]
            my0_all[:, :, l] = m0 * (1.0 - wx)[..., None, None]
            my1_all[:, :, l] = m1 * wx[..., None, None]

    f1r = fmap1.reshape(B, D, H, W)

    def core_map(c):
        b = c // (NCORES // B)
        # f1 columns: [k, 128p(D), NT*P queries] in (t, p) order
        f1c = f1r[b][:, _HH[c].reshape(-1), _WW[c].reshape(-1)]  # [D, QPC]
        return {
            "f1t": np.ascontiguousarray(f1c.reshape(2, P, QPC)).astype(bf16),
            "f2t": np.ascontiguousarray(f2cat[b].reshape(2, P, NPOS)),
            # [P, NT*NLVL] with col = t*NLVL + l
            "idxt": np.ascontiguousarray(
                idx_all[c].reshape(NT * NLVL, P).T
            ),
            # [P, (l*NT+t)*2]
            "wgtt": np.ascontiguousarray(
                wgt_all[c].transpose(2, 1, 0, 3).reshape(P, -1)
            ),
            "my0t": np.ascontiguousarray(
                my0_all[c].transpose(1, 0, 2, 3, 4).reshape(NLVL * NT, P, 90)
                .transpose(1, 0, 2).reshape(P, -1)
            ),
            "my1t": np.ascontiguousarray(
                my1_all[c].transpose(1, 0, 2, 3, 4).reshape(NLVL * NT, P, 90)
                .transpose(1, 0, 2).reshape(P, -1)
            ),
        }

    return [core_map(c) for c in range(NCORES)]


def assemble(results):
    out = np.empty((B, NCH, H, W), np.float32)
    for c in range(NCORES):
        r = np.asarray(results[c]["outp"], np.float32)  # [NT, P, NCH]
        for t in range(NT):
            blk = r[t].T.reshape(NCH, BH, BW)  # [NCH, 4, 32]
            b, h0, w0 = block_of(c, t)
            for l in range(NLVL):
                sec = blk[l * 81 : (l + 1) * 81]
                if not XMAJ[l]:
                    sec = sec.reshape(9, 9, BH, BW).transpose(1, 0, 2, 3).reshape(
                        81, BH, BW
                    )
                out[b, l * 81 : (l + 1) * 81, h0 : h0 + BH, w0 : w0 + BW] = sec
    return out


_NC_CACHE = {}


def get_nc(slabs, **kw):
    key = (slabs, tuple(sorted(kw.items())))
    if key not in _NC_CACHE:
        _NC_CACHE[key] = build_nc(slabs, **kw)
    return _NC_CACHE[key]


def kernel(fmap1, fmap2, coords):
    slabs = compute_slabs(coords)
    in_maps = _host_prep(fmap1, fmap2, coords, slabs)
    nc = get_nc(slabs)
    res = run_bass_kernel_spmd(nc, in_maps, core_ids=list(range(NCORES)))
    return assemble(res.results)
